# revision 1
# baseline (speedup 1.0000x reference)
"""Trainium2 Bass kernel for nn_Block_46995532153006 (dense transformer block
with YatDense layers, causal attention, gated MLP).

Sharding: 8 cores = (batch b in {0,1}) x (seq-group g in {0..3}).
Core 4b+g owns row-blocks {g, 7-g} (128 rows each) of batch b.  All layers are
row-parallel; the only collective is one AllGather of (K^T, V-hat) per batch
group before attention.  Causal attention uses balanced block pairs plus
uniform padded j-loops (rt0: 4 key blocks, rt1: 8) with per-core data masks so
the single SPMD program is identical across cores.

Matmuls run in bf16 with fp32 PSUM accumulation; the YatDense epilogue
  out = scale * y^2 / (||x||^2 + ||w||^2 - 2y + eps) + scale*b
uses host-precomputed column norms (cnb) and scale*bias (sbb) broadcast tiles.
PV uses a "V-hat" layout [keys, 66] per head (V, a ones column, zero pad) so
the softmax denominator falls out of the same matmul accumulation.

Hardware constraints honored here (found the hard way):
- matmuls with different PE tile_position (partition offset 0 vs 64) must not
  target the same PSUM bank -> attention heads are grouped by parity.
- only one open PSUM accumulation group per bank -> the PV accumulation over
  key blocks is emitted as a single start/stop group spanning all head slots.
"""

import math
from contextlib import ExitStack
import numpy as np
import ml_dtypes
import sys

sys.path.insert(0, "/opt/trn_rl_repo")

import concourse.bass as bass
import concourse.bacc as bacc
import concourse.mybir as mybir
import concourse.tile as tile
from concourse import masks as cmasks
from concourse import bass_utils

BF16 = mybir.dt.bfloat16
F32 = mybir.dt.float32
ALU = mybir.AluOpType
ACT = mybir.ActivationFunctionType
NPBF = ml_dtypes.bfloat16

B, T, C, H = 2, 1024, 768, 12
D = C // H          # 64
HID = 4 * C         # 3072
P = 128
NBLK = T // P       # 8 row blocks per batch
EPS = 1e-6
NB = 384            # matmul free-dim tile (psum tiles [128, 384])
VW = 66             # V-hat slot width: 64 V + 1 ones + 1 pad (8B aligned)
KVROW = 768 + H * VW

_CACHE = {}
LAST_RES = None


def _build(scales, use_bias, use_g1, use_g2):
    sc_qkv, sc_ao, sc_fc, sc_gate, sc_proj = scales
    nc = bacc.Bacc("TRN2", target_bir_lowering=False, debug=False,
                   num_devices=8)

    def din(name, shape, dt):
        return nc.dram_tensor(name, list(shape), dt, kind="ExternalInput").ap()

    x_d = din("x_own", (2, P, C), F32)
    rope_d = din("rope_own", (2, P, C), BF16)
    msk_d = din("mask_own", (2, NBLK, P, P), BF16)
    wq_d = din("w_qkv", (C, 3 * C), BF16)
    wao_d = din("w_ao", (C, C), BF16)
    wfc_d = din("w_fc", (C, HID), BF16)
    wg_d = din("w_gate", (C, HID), BF16)
    wp_d = din("w_proj", (HID, C), BF16)
    cnb_d = {
        "qkv": din("cnb_qkv", (P, 3 * C), BF16),
        "ao": din("cnb_ao", (P, C), BF16),
        "fc": din("cnb_fc", (P, HID), BF16),
        "gate": din("cnb_gate", (P, HID), BF16),
        "proj": din("cnb_proj", (P, C), BF16),
    }
    sbb_d = {}
    if use_bias:
        sbb_d = {
            "qkv": din("sbb_qkv", (P, 3 * C), BF16),
            "ao": din("sbb_ao", (P, C), BF16),
            "fc": din("sbb_fc", (P, HID), BF16),
            "gate": din("sbb_gate", (P, HID), BF16),
            "proj": din("sbb_proj", (P, C), BF16),
        }
    gb1_d = din("gb1", (P, C), F32) if use_g1 else None
    gb2_d = din("gb2", (P, C), F32) if use_g2 else None
    y_d = nc.dram_tensor("y_own", [2, P, C], F32, kind="ExternalOutput").ap()

    # internal DRAM for the collective
    kv_loc = nc.dram_tensor("kv_loc", [2, P, KVROW], BF16).ap()
    kv_gth = nc.dram_tensor("kv_gth", [NBLK, P, KVROW], BF16).ap()

    # gathered index of key-block j (blocks {g, 7-g} per rank, in rank order)
    def gidx(j):
        return 2 * j if j < 4 else 2 * (7 - j) + 1

    with tile.TileContext(nc) as tc, ExitStack() as ctx:
        cp = ctx.enter_context(tc.tile_pool(name="consts", bufs=1))
        wpl = ctx.enter_context(tc.tile_pool(name="wpool", bufs=26))
        lcp = ctx.enter_context(tc.tile_pool(name="layerconst", bufs=3))
        ep = ctx.enter_context(tc.tile_pool(name="epi", bufs=2))
        sp = ctx.enter_context(tc.tile_pool(name="small", bufs=6))
        scp = ctx.enter_context(tc.tile_pool(name="scratch", bufs=2))
        pers = ctx.enter_context(tc.tile_pool(name="pers", bufs=1))
        ptp = ctx.enter_context(tc.tile_pool(name="ptpool", bufs=3))
        pp = {}

        def TL(pool, shape, dt, tag):
            return pool.tile(shape, dt, name=tag, tag=tag)

        # ---- constants ----
        ident = TL(cp, [P, P], BF16, "ident")
        cmasks.make_identity(nc, ident[:])
        zb = TL(cp, [P, 1], F32, "zb")
        nc.gpsimd.memset(zb[:], 0.0)
        x_sb = TL(cp, [P, 2, C], F32, "x_sb")
        nc.sync.dma_start(out=x_sb[:], in_=x_d.rearrange("r p f -> p r f"))
        rope_sb = TL(cp, [P, 2, C], BF16, "rope_sb")
        nc.sync.dma_start(out=rope_sb[:],
                          in_=rope_d.rearrange("r p f -> p r f"))
        msk_sb = TL(cp, [P, 2, NBLK, P], BF16, "msk_sb")
        nc.sync.dma_start(out=msk_sb[:],
                          in_=msk_d.rearrange("r j p f -> p r j f"))
        gb1 = gb2 = None
        if use_g1:
            gb1 = TL(cp, [P, C], F32, "gb1")
            nc.sync.dma_start(out=gb1[:], in_=gb1_d)
        if use_g2:
            gb2 = TL(cp, [P, C], F32, "gb2")
            nc.sync.dma_start(out=gb2[:], in_=gb2_d)

        def load_w_chunks(wd, n_in, n_out):
            """load weight [n_in, n_out] as [n_in/128] x [128, 768] tiles"""
            tiles = []
            for kc in range(n_in // P):
                row = []
                for cc in range(n_out // C):
                    t = TL(wpl, [P, C], BF16, "w")
                    nc.sync.dma_start(
                        out=t[:], in_=wd[P * kc:P * kc + P, C * cc:C * cc + C])
                    row.append(t)
                tiles.append(row)
            return tiles

        def load_cnb(key, n):
            t = lcp.tile([P, n], BF16, name="cnb", tag="cnb",
                         bufs=2 if (use_bias or use_g1 or use_g2) else 3)
            nc.sync.dma_start(out=t[:], in_=cnb_d[key])
            s = None
            if use_bias:
                s = lcp.tile([P, n], BF16, name="sbb", tag="sbb", bufs=1)
                nc.sync.dma_start(out=s[:], in_=sbb_d[key])
            return t, s

        # ---- helpers ----
        def layernorm(x_ap, gb, out_bf, rn_out):
            red = TL(sp, [P, 1], F32, "red")
            nc.vector.tensor_reduce(red[:], x_ap, mybir.AxisListType.X,
                                    ALU.add)
            mu = TL(sp, [P, 1], F32, "mu")
            nc.vector.tensor_scalar_mul(mu[:], red[:], 1.0 / C)
            xc = TL(ep, [P, C], F32, "xc")
            nc.vector.tensor_scalar(xc[:], x_ap, mu[:], None, ALU.subtract)
            scr = TL(scp, [P, HID], BF16, "scr")
            ssq = TL(sp, [P, 1], F32, "ssq")
            nc.scalar.activation(scr[:, 0:C], xc[:], ACT.Square, bias=zb[:],
                                 accum_out=ssq[:])
            var = TL(sp, [P, 1], F32, "var")
            nc.vector.tensor_scalar(var[:], ssq[:], 1.0 / C, EPS, ALU.mult,
                                    ALU.add)
            sd = TL(sp, [P, 1], F32, "sd")
            nc.scalar.activation(sd[:], var[:], ACT.Sqrt, bias=zb[:])
            rstd = TL(sp, [P, 1], F32, "rstd")
            nc.vector.reciprocal(rstd[:], sd[:])
            if gb is not None:
                nc.vector.scalar_tensor_tensor(out_bf, xc[:], rstd[:], gb[:],
                                               ALU.mult, ALU.mult)
            else:
                nc.vector.tensor_scalar(out_bf, xc[:], rstd[:], None, ALU.mult)
            scr2 = TL(scp, [P, HID], BF16, "scr")
            nc.scalar.activation(scr2[:, 0:C], out_bf, ACT.Square, bias=zb[:],
                                 accum_out=rn_out)

        def transpose_to(dst_ap, src_ap, use_act):
            """dst[128,128] = src[128,128].T via PE (+psum copy)"""
            pt = TL(pp["tp"], [P, P], BF16, "tp")
            nc.tensor.transpose(pt[:], src_ap, ident[:])
            if use_act:
                nc.scalar.copy(dst_ap, pt[:])
            else:
                nc.vector.tensor_copy(dst_ap, pt[:])

        def yat_epi(psum_ap, rn_ap, cnb_ap, sbb_ap, scale, dest_ap):
            """dest = scale*psum^2 / (rn - 2*psum + cnb) [+ sbb]"""
            y2 = TL(ep, [P, NB], F32, "y2")
            nc.scalar.activation(y2[:], psum_ap, ACT.Square, bias=zb[:])
            d = TL(ep, [P, NB], F32, "d")
            nc.vector.tensor_scalar(d[:], psum_ap, -2.0, rn_ap, ALU.mult,
                                    ALU.add)
            nc.vector.tensor_tensor(d[:], d[:], cnb_ap, ALU.add)
            r = TL(ep, [P, NB], F32, "r")
            nc.vector.reciprocal_approx_fast(r[:], d[:])
            if sbb_ap is None:
                nc.vector.scalar_tensor_tensor(dest_ap, y2[:], float(scale),
                                               r[:], ALU.mult, ALU.mult)
            else:
                nc.vector.scalar_tensor_tensor(y2[:], y2[:], float(scale),
                                               r[:], ALU.mult, ALU.mult)
                nc.vector.tensor_tensor(dest_ap, y2[:], sbb_ap, ALU.add)

        def dense_yat(key, w_tiles, n_in, n_out, lhsT, rn, scale, dest_fn):
            """dest = yat(lhsT.T @ W).  lhsT[rt] = [128, n_in/128, 128] tile.
            dest_fn(rt, nb) -> dest AP [128, NB]."""
            cnb, sbb = load_cnb(key, n_out)
            nkc = n_in // P
            for nb in range(n_out // NB):
                for rt in range(2):
                    ps = TL(pp["mm"], [P, NB], F32, "mm")
                    cc, off = (NB * nb) // C, (NB * nb) % C
                    for kc in range(nkc):
                        nc.tensor.matmul(
                            ps[:], lhsT[rt][:, kc, :],
                            w_tiles[kc][cc][:, off:off + NB],
                            start=(kc == 0), stop=(kc == nkc - 1))
                    yat_epi(ps[:], rn[rt][:], cnb[:, NB * nb:NB * nb + NB],
                            None if sbb is None
                            else sbb[:, NB * nb:NB * nb + NB],
                            scale, dest_fn(rt, nb))

        # persistent activation tiles
        # xT[rt] serves sequentially as: h1T -> QT -> oT -> h2T
        xT = [TL(pers, [P, 6, P], BF16, f"xT_{rt}") for rt in range(2)]
        # h1[rt] doubles as q_pre after rn1/h1T are consumed
        h1 = [TL(pers, [P, C], BF16, f"h1_{rt}") for rt in range(2)]
        rn1 = [TL(pers, [P, 1], F32, f"rn1_{rt}") for rt in range(2)]
        vh = [TL(pers, [P, H, VW], BF16, f"vh_{rt}") for rt in range(2)]
        k_pre = [TL(pers, [P, C], BF16, f"kpre_{rt}") for rt in range(2)]

        # =================================================================
        # Phase 1: LN1 + QKV + rope + transposes + V-hat, per rt
        # =================================================================
        with tc.tile_pool(name="psmm1", bufs=3, space="PSUM") as _mm, \
                tc.tile_pool(name="pstp1", bufs=2, space="PSUM") as _tp:
            pp["mm"], pp["tp"] = _mm, _tp

            for rt in range(2):
                layernorm(x_sb[:, rt, :], gb1, h1[rt][:], rn1[rt][:])
                for kc in range(6):
                    transpose_to(xT[rt][:, kc, :],
                                 h1[rt][:, P * kc:P * kc + P],
                                 use_act=(kc % 2 == 0))
                nc.vector.memset(vh[rt][:, :, 64:65], 1.0)
                nc.vector.memset(vh[rt][:, :, 65:66], 0.0)

            wq_tiles = load_w_chunks(wq_d, C, 3 * C)

            def qkv_dest(rt, nb):
                if nb < 2:
                    return h1[rt][:, NB * nb:NB * nb + NB]          # q_pre
                if nb < 4:
                    return k_pre[rt][:, NB * (nb - 2):NB * (nb - 2) + NB]
                a = 6 * (nb - 4)
                return vh[rt][:, a:a + 6, 0:64]

            dense_yat("qkv", wq_tiles, C, 3 * C, xT, rn1, sc_qkv, qkv_dest)

            # rope on q/k (in place), then transposes; KT/V-hat go to DRAM
            for rt in range(2):
                nc.vector.tensor_tensor(h1[rt][:], h1[rt][:],
                                        rope_sb[:, rt, :], ALU.mult)
                nc.vector.tensor_tensor(k_pre[rt][:], k_pre[rt][:],
                                        rope_sb[:, rt, :], ALU.mult)
                ktl = TL(pers, [P, 6, P], BF16, f"ktl_{rt}")
                for kc in range(6):
                    transpose_to(xT[rt][:, kc, :],
                                 h1[rt][:, P * kc:P * kc + P],
                                 use_act=(kc % 2 == 1))             # QT
                    transpose_to(ktl[:, kc, :],
                                 k_pre[rt][:, P * kc:P * kc + P],
                                 use_act=(kc % 2 == 0))
                nc.sync.dma_start(out=kv_loc[rt, :, 0:768],
                                  in_=ktl[:].rearrange("p a b -> p (a b)"))
                nc.sync.dma_start(out=kv_loc[rt, :, 768:KVROW],
                                  in_=vh[rt][:].rearrange("p a b -> p (a b)"))

        # =================================================================
        # Phase 2: AllGather KT/V-hat within batch group
        # =================================================================
        nc.gpsimd.collective_compute(
            "AllGather", ALU.bypass,
            replica_groups=[[0, 1, 2, 3], [4, 5, 6, 7]],
            ins=[kv_loc.opt()], outs=[kv_gth.opt()])

        ktg = TL(pers, [P, NBLK, C], BF16, "ktg")
        vhg = TL(pers, [P, NBLK, H * VW], BF16, "vhg")
        for gi in range(NBLK):
            nc.sync.dma_start(out=ktg[:, gi, :], in_=kv_gth[gi, :, 0:768])
            nc.sync.dma_start(out=vhg[:, gi, :], in_=kv_gth[gi, :, 768:KVROW])
        vhg3 = vhg[:].rearrange("p g (h v) -> p g h v", h=H)

        # =================================================================
        # Phase 3: attention (S^T = K @ Q^T; PV via V-hat; parity groups)
        # =================================================================
        o_nat = [TL(pers, [P, C], BF16, f"onat_{rt}") for rt in range(2)]
        with tc.tile_pool(name="psst", bufs=2, space="PSUM") as ps_st, \
                tc.tile_pool(name="pso6", bufs=2, space="PSUM") as ps_o6:
            for rt in range(2):
                njs = 4 if rt == 0 else NBLK
                for par in range(2):
                    heads = [2 * s + par for s in range(6)]
                    off = par * 64
                    po = TL(ps_o6, [P, 6 * VW], F32, "po")
                    for j in range(njs):
                        gi = gidx(j)
                        pst = TL(ps_st, [P, 6 * P], F32, "pst")
                        for s, hh in enumerate(heads):
                            kc = hh // 2
                            nc.tensor.matmul(
                                pst[:, P * s:P * s + P],
                                ktg[off:off + 64, gi, P * kc:P * kc + P],
                                xT[rt][off:off + 64, kc, :],
                                start=True, stop=True)
                        pt = TL(ptp, [P, 6 * P], BF16, "pt")
                        nc.scalar.activation(pt[:], pst[:], ACT.Exp,
                                             bias=zb[:],
                                             scale=1.0 / math.sqrt(D))
                        ptv = pt[:].rearrange("p (s f) -> p s f", s=6)
                        mb = msk_sb[:, rt, j:j + 1, :].broadcast_to([P, 6, P])
                        nc.vector.tensor_tensor(ptv, ptv, mb, ALU.mult)
                        for s, hh in enumerate(heads):
                            nc.tensor.matmul(
                                po[:, VW * s:VW * s + VW],
                                pt[:, P * s:P * s + P],
                                vhg3[:, gi, hh, :],
                                start=(j == 0 and s == 0),
                                stop=(j == njs - 1 and s == 5))
                    for s, hh in enumerate(heads):
                        rd = TL(sp, [P, 1], F32, "rd")
                        nc.vector.reciprocal(
                            rd[:], po[:, VW * s + 64:VW * s + 65])
                        nc.vector.tensor_scalar(
                            o_nat[rt][:, 64 * hh:64 * hh + 64],
                            po[:, VW * s:VW * s + 64], rd[:], None, ALU.mult)

        # =================================================================
        # Phases 4+5: attention c_proj + residual + MLP
        # =================================================================
        with tc.tile_pool(name="psmm2", bufs=3, space="PSUM") as _mm2, \
                tc.tile_pool(name="pstp2", bufs=2, space="PSUM") as _tp2:
            pp["mm"], pp["tp"] = _mm2, _tp2

            rn_o = [TL(pers, [P, 1], F32, f"rno_{rt}") for rt in range(2)]
            for rt in range(2):
                scr = TL(scp, [P, HID], BF16, "scr")
                nc.scalar.activation(scr[:, 0:C], o_nat[rt][:], ACT.Square,
                                     bias=zb[:], accum_out=rn_o[rt][:])
                for kc in range(6):
                    transpose_to(xT[rt][:, kc, :],                    # oT
                                 o_nat[rt][:, P * kc:P * kc + P],
                                 use_act=(kc % 2 == 0))

            wao_tiles = load_w_chunks(wao_d, C, C)
            o_prj = [TL(pers, [P, C], BF16, f"oprj_{rt}") for rt in range(2)]
            dense_yat("ao", wao_tiles, C, C, xT, rn_o, sc_ao,
                      lambda rt, nb: o_prj[rt][:, NB * nb:NB * nb + NB])

            x1 = [TL(pers, [P, C], F32, f"x1_{rt}") for rt in range(2)]
            for rt in range(2):
                nc.vector.tensor_tensor(x1[rt][:], x_sb[:, rt, :],
                                        o_prj[rt][:], ALU.add)

            rn2 = [TL(pers, [P, 1], F32, f"rn2_{rt}") for rt in range(2)]
            for rt in range(2):
                h2 = TL(ep, [P, C], BF16, "h2")
                layernorm(x1[rt][:], gb2, h2[:], rn2[rt][:])
                for kc in range(6):
                    transpose_to(xT[rt][:, kc, :], h2[:, P * kc:P * kc + P],
                                 use_act=(kc % 2 == 1))              # h2T

            gate = [TL(pers, [P, HID], BF16, f"gate_{rt}") for rt in range(2)]
            u = [TL(pers, [P, HID], BF16, f"u_{rt}") for rt in range(2)]
            wfc_tiles = load_w_chunks(wfc_d, C, HID)
            dense_yat("fc", wfc_tiles, C, HID, xT, rn2, sc_fc,
                      lambda rt, nb: gate[rt][:, NB * nb:NB * nb + NB])
            wg_tiles = load_w_chunks(wg_d, C, HID)
            dense_yat("gate", wg_tiles, C, HID, xT, rn2, sc_gate,
                      lambda rt, nb: u[rt][:, NB * nb:NB * nb + NB])

            rn_m = [TL(pers, [P, 1], F32, f"rnm_{rt}") for rt in range(2)]
            mT = []
            for rt in range(2):
                nc.scalar.activation(u[rt][:], u[rt][:], ACT.Gelu_apprx_tanh,
                                     bias=zb[:])
                nc.vector.tensor_tensor(gate[rt][:], gate[rt][:], u[rt][:],
                                        ALU.mult)
                scr = TL(scp, [P, HID], BF16, "scr")
                nc.scalar.activation(scr[:], gate[rt][:], ACT.Square,
                                     bias=zb[:], accum_out=rn_m[rt][:])
                # transpose m into the (now dead) u tile, viewed [P, 24, P]
                mTv = u[rt][:].rearrange("p (a b) -> p a b", a=24)
                mT.append(mTv)
                for kc in range(24):
                    transpose_to(mTv[:, kc, :],
                                 gate[rt][:, P * kc:P * kc + P],
                                 use_act=(kc % 2 == 0))

            wpj_tiles = load_w_chunks(wp_d, HID, C)
            p_out = [TL(pers, [P, C], BF16, f"pout_{rt}") for rt in range(2)]
            dense_yat("proj", wpj_tiles, HID, C, mT, rn_m, sc_proj,
                      lambda rt, nb: p_out[rt][:, NB * nb:NB * nb + NB])

            for rt in range(2):
                of = TL(ep, [P, C], F32, "of")
                nc.vector.tensor_tensor(of[:], x1[rt][:], p_out[rt][:],
                                        ALU.add)
                nc.sync.dma_start(out=y_d[rt], in_=of[:])

    nc.compile()
    return nc


# --------------------------------------------------------------------------
# host side
# --------------------------------------------------------------------------

def _rope_full():
    freqs = np.exp(np.arange(0, D, 2, dtype=np.float32)
                   * (-np.log(10000.0) / D))
    ang = np.arange(T, dtype=np.float32)[:, None] * freqs[None, :]
    r = np.concatenate([np.cos(ang), np.sin(ang)], -1)   # [T, D]
    return np.tile(r, (1, H)).astype(np.float32)         # [T, C]


def _prepare(**inputs):
    inp = {k: np.asarray(v) for k, v in inputs.items()}
    x = inp["x"].astype(np.float32)
    w = {k: np.asarray(v, np.float32) for k, v in inp.items()
         if k not in ("x", "mask")}

    def cn_sb(wn, bn, an):
        W = w[wn]
        n = W.shape[1]
        cn = (W ** 2).sum(0) + EPS
        scale = (np.sqrt(np.float32(n)) / np.log1p(np.float32(n))) \
            ** float(np.asarray(w[an]).reshape(-1)[0])
        return cn.astype(np.float32), (scale * w[bn]).astype(np.float32), \
            float(scale)

    cn_qkv, sb_qkv, sc_qkv = cn_sb("w_qkv", "b_qkv", "a_qkv")
    cn_ao, sb_ao, sc_ao = cn_sb("w_ao", "b_ao", "a_ao")
    cn_fc, sb_fc, sc_fc = cn_sb("w_fc", "b_fc", "a_fc")
    cn_gate, sb_gate, sc_gate = cn_sb("w_gate", "b_gate", "a_gate")
    cn_proj, sb_proj, sc_proj = cn_sb("w_proj", "b_proj", "a_proj")

    use_bias = any(np.any(w[b] != 0.0)
                   for b in ("b_qkv", "b_ao", "b_fc", "b_gate", "b_proj"))
    use_g1 = bool(np.any(w["ln1_scale"] != 1.0))
    use_g2 = bool(np.any(w["ln2_scale"] != 1.0))
    scales = (sc_qkv, sc_ao, sc_fc, sc_gate, sc_proj)

    key = (scales, use_bias, use_g1, use_g2)
    if key not in _CACHE:
        _CACHE[key] = _build(scales, use_bias, use_g1, use_g2)
    nc = _CACHE[key]

    rope = _rope_full()

    def bcast(a):
        return np.ascontiguousarray(np.broadcast_to(a[None, :],
                                                    (P, a.shape[0])))

    shared = {
        "w_qkv": w["w_qkv"].astype(NPBF), "w_ao": w["w_ao"].astype(NPBF),
        "w_fc": w["w_fc"].astype(NPBF), "w_gate": w["w_gate"].astype(NPBF),
        "w_proj": w["w_proj"].astype(NPBF),
        "cnb_qkv": bcast(cn_qkv).astype(NPBF),
        "cnb_ao": bcast(cn_ao).astype(NPBF),
        "cnb_fc": bcast(cn_fc).astype(NPBF),
        "cnb_gate": bcast(cn_gate).astype(NPBF),
        "cnb_proj": bcast(cn_proj).astype(NPBF),
    }
    if use_bias:
        shared.update({
            "sbb_qkv": bcast(sb_qkv).astype(NPBF),
            "sbb_ao": bcast(sb_ao).astype(NPBF),
            "sbb_fc": bcast(sb_fc).astype(NPBF),
            "sbb_gate": bcast(sb_gate).astype(NPBF),
            "sbb_proj": bcast(sb_proj).astype(NPBF),
        })
    if use_g1:
        shared["gb1"] = bcast(w["ln1_scale"]).astype(np.float32)
    if use_g2:
        shared["gb2"] = bcast(w["ln2_scale"]).astype(np.float32)

    in_maps = []
    for core in range(8):
        b, g = core // 4, core % 4
        blks = (g, 7 - g)
        x_own = np.stack([x[b, P * bl:P * bl + P] for bl in blks])
        rope_own = np.stack([rope[P * bl:P * bl + P] for bl in blks]) \
            .astype(NPBF)
        mask_own = np.zeros((2, NBLK, P, P), NPBF)
        for rt, bl in enumerate(blks):
            qglob = P * bl + np.arange(P)
            for j in range(NBLK):
                kglob = P * j + np.arange(P)
                mask_own[rt, j] = (kglob[:, None]
                                   <= qglob[None, :]).astype(NPBF)
        m = dict(shared)
        m["x_own"] = np.ascontiguousarray(x_own).astype(np.float32)
        m["rope_own"] = np.ascontiguousarray(rope_own)
        m["mask_own"] = mask_own
        in_maps.append(m)

    return nc, in_maps


def _assemble(results):
    out = np.zeros((B, T, C), np.float32)
    for core in range(8):
        b, g = core // 4, core % 4
        y = results[core]["y_own"]
        for rt, bl in enumerate((g, 7 - g)):
            out[b, P * bl:P * bl + P] = y[rt]
    return out


def kernel(**inputs):
    global LAST_RES
    nc, in_maps = _prepare(**inputs)
    res = bass_utils.run_bass_kernel_spmd(nc, in_maps,
                                          core_ids=list(range(8)))
    LAST_RES = res
    return _assemble(res.results)


def _run_fast(nc, in_maps, iters=10):
    """Execute with device-resident inputs; returns (results, min_exec_ns).
    Mirrors bass2jax.run_bass_via_pjrt but keeps the jitted fn + inputs on
    device so repeated executions measure dispatch+execute only."""
    import time
    import jax
    from jax.sharding import Mesh, PartitionSpec, NamedSharding
    try:
        from jax.experimental.shard_map import shard_map
    except ImportError:
        from jax.shard_map import shard_map
    from concourse.bass2jax import (_bass_exec_p, install_neuronx_cc_hook,
                                    partition_id_tensor)

    install_neuronx_cc_hook()
    n_cores = len(in_maps)
    in_names, out_names, out_avals, zero_outs = [], [], [], []
    for alloc in nc.m.functions[0].allocations:
        if not isinstance(alloc, mybir.MemoryLocationSet):
            continue
        name = alloc.memorylocations[0].name
        if alloc.kind == "ExternalInput":
            if nc.partition_id_tensor is None or \
                    name != nc.partition_id_tensor.name:
                in_names.append(name)
        elif alloc.kind == "ExternalOutput":
            out_names.append(name)
            shape = tuple(alloc.tensor_shape)
            dtype = mybir.dt.np(alloc.dtype)
            out_avals.append(jax.core.ShapedArray(shape, dtype))
            zero_outs.append(np.zeros(shape, dtype))
    n_params = len(in_names)
    n_outs = len(out_avals)
    all_names = in_names + out_names
    if nc.partition_id_tensor is not None:
        all_names = all_names + [nc.partition_id_tensor.name]

    def _body(*args):
        operands = list(args)
        if nc.partition_id_tensor is not None:
            operands.append(partition_id_tensor())
        return tuple(_bass_exec_p.bind(
            *operands, out_avals=tuple(out_avals), in_names=tuple(all_names),
            out_names=tuple(out_names), lowering_input_output_aliases=(),
            sim_require_finite=True, sim_require_nnan=True, nc=nc))

    devices = jax.devices()[:n_cores]
    mesh = Mesh(np.asarray(devices), ("core",))
    sharded = jax.jit(
        shard_map(_body, mesh=mesh,
                  in_specs=(PartitionSpec("core"),) * (n_params + n_outs),
                  out_specs=(PartitionSpec("core"),) * n_outs,
                  check_rep=False),
        keep_unused=True)
    sh = NamedSharding(mesh, PartitionSpec("core"))
    concat_in = [
        jax.device_put(
            np.concatenate([np.asarray(in_maps[c][n])
                            for c in range(n_cores)], axis=0), sh)
        for n in in_names
    ]
    concat_zeros = [
        jax.device_put(np.zeros((n_cores * z.shape[0], *z.shape[1:]),
                                z.dtype), sh)
        for z in zero_outs
    ]
    out_arrs = sharded(*concat_in, *concat_zeros)
    jax.block_until_ready(out_arrs)
    results = [
        {name: np.asarray(out_arrs[i]).reshape(n_cores,
                                               *out_avals[i].shape)[c]
         for i, name in enumerate(out_names)}
        for c in range(n_cores)
    ]
    best = None
    for _ in range(iters):
        t0 = time.perf_counter()
        out_arrs = sharded(*concat_in, *concat_zeros)
        jax.block_until_ready(out_arrs)
        dt = time.perf_counter() - t0
        best = dt if best is None or dt < best else best
    return results, int(best * 1e9)


def bench(iters=10, **inputs):
    """Run the kernel with a timed loop; returns (full_output, min_exec_ns).
    Note: on this axon-tunneled setup the per-dispatch overhead floor is
    ~40-80 ms, which dominates the measured time; the device-side kernel
    span itself is far smaller."""
    nc, in_maps = _prepare(**inputs)
    results, ns = _run_fast(nc, in_maps, iters=iters)
    return _assemble(results), ns

